# revision 23
# baseline (speedup 1.0000x reference)
"""Multi-head attention (B=2, S=2048, D=1024, H=16) on 8 TRN2 NeuronCores.

Active variant (build_mha_v4): sequence-data-parallel with K/V AllGather.
Core c handles batch b = c // 4 and sequence chunk j = c % 4 (512 rows,
used both as its query block and as its K/V contribution).  Each core
projects K/V/Q only for its own 512 rows (3x less projection work than
computing K/V redundantly); K and V chunks are AllGather'd through HBM
bounce buffers within the 4-core batch group (replica groups [[0..3],
[4..7]], ~27 us per 4MB gather, overlapped with the remaining
projections).  Attention runs per head in scores.T [s, q] orientation
(exp on ACT with no max subtraction -- scores are ~N(0, 0.41)); softmax
denominators come free from a ones-column appended to V.  PV runs
transposed (es stationary, V moving, cost 65 rows instead of 512 per
s-tile) yielding A[q, d] plus the denominator column in PSUM; the
reciprocal denominator multiplies during eviction, and a PE transpose
returns A to [d, q] layout for the output projection.  Engine split: ACT
does only the 128 exp ops (the ~127 us floor), DVE does reciprocal/
normalize/evictions, Pool only triggers collectives (GPSIMD cannot touch
PSUM), PE does all matmuls/transposes.

Older zero-collective variants (build_mha/build_mha_v2) are kept for
reference and A/B timing.  Host-side prep passes operands pre-transposed
([in_dim, out_dim], contraction on partitions) and pre-rounded to bf16;
on-device accumulation is fp32 PSUM.

Collectives cannot live inside a For_i hardware loop (mesh desync), so
the reps>1 timing variant of v4 unrolls the body instead.
"""

import sys

for _p in ("/opt/trn_rl_repo",):
    if _p not in sys.path:
        sys.path.insert(0, _p)

import numpy as np
import ml_dtypes

import bass_rust
import concourse.bass as bass
import concourse.mybir as mybir
import concourse.tile as tile
from concourse.vector_clock import ScopedClock, VectorClock

F32 = mybir.dt.float32
F32R = mybir.dt.float32r
BF16 = mybir.dt.bfloat16
AF = mybir.ActivationFunctionType

D = 1024
S = 2048
SQ = 512
H = 16
DK = 64
NT_D = D // 128
NT_S = S // 128
NT_Q = SQ // 128
N_CORES = 8

# ---------------------------------------------------------------------------
# Workarounds for this walrus build, which accepts at most ONE semaphore wait
# per instruction ('Too many sync wait commands' in setupSyncWait).  Tile
# attaches multiple waits freely; split them across same-engine nops, and
# emit the kernel-tail drain one waited-semaphore at a time.
# ---------------------------------------------------------------------------

_WAITS_PER_INST = 1


def _split_drain_and_barrier(self, tick_clock, wait_clock):
    gc = tick_clock.global_clock
    n = len(gc)
    procs = [i for i in range(n) if gc[i] > 0]
    for i in range(0, len(procs), _WAITS_PER_INST):
        group = procs[i : i + _WAITS_PER_INST]
        vec = [0] * n
        for p in group:
            vec[p] = gc[p]
        drain_inst = self.nc.sync.drain()
        wait_clock.add_sem_waits(drain_inst.ins, ScopedClock({None: VectorClock(vec)}))

    self.nc.all_engine_barrier()
    assert self.sems is not None
    popped = self.nc._tile_sem_poison_stack.pop()
    assert popped is self._sem_poison
    self.nc.clear_and_free_semaphores(list(self.sems.allocated().values()))
    self.nc.all_engine_barrier()


tile.TileContext._drain_and_barrier = _split_drain_and_barrier


def _split_sync_waits(nc, limit=_WAITS_PER_INST):
    for f in nc.m.functions:
        for bb in f.blocks:
            insts = list(bb.instructions)
            if not any(
                inst.sync_info and len(inst.sync_info.on_wait or []) > limit
                for inst in insts
            ):
                continue
            new_list = []
            for inst in insts:
                si = inst.sync_info
                waits = list(si.on_wait) if si and si.on_wait else []
                if len(waits) > limit:
                    extra, keep = waits[:-limit], waits[-limit:]
                    for j in range(0, len(extra), limit):
                        chunk = extra[j : j + limit]
                        nop = nc.engines[inst.engine].nop(nofuse=True).ins
                        cur = nc.cur_bb.bb
                        assert cur.instructions[-1].name == nop.name
                        cur.instructions.pop()
                        nop.sync_info = bass_rust.SyncInfo(on_wait=chunk, on_update=[])
                        new_list.append(nop)
                    si.on_wait = keep
                new_list.append(inst)
            bb.instructions[:] = new_list


# ---------------------------------------------------------------------------
# v4: all-gather K/V builder
# ---------------------------------------------------------------------------
#
# Core c = (b, j): batch b = c//4, chunk j = c%4 owns sequence rows
# [512j, 512j+512) both as queries and as K/V rows.  Each core projects
# K/V/Q only for its own 512 rows; K and V chunks are AllGather'd (HBM
# bounce) within the 4-core batch group, giving full-S K/V with zero
# redundant projection work.  Attention runs per head with scores.T [s, q]
# (exp on ACT, no max subtraction), PV in transposed form (es stationary,
# V moving) yielding A[q, d] + denominator column; normalization multiplies
# by the reciprocal denominator during PSUM eviction, then a PE transpose
# puts A back in [d, q] layout for the output projection.
#
# Engine budget per core (rows = moving-row cycles at 128x128):
#   PE : 3x32768 proj + 131072 QK + 66560 PV + 8192 transp + 32768 Oproj
#   ACT: 128 exp ops of [128, 1024]  (the ~127 us floor; nothing else)
#   Pool: evictions w/ bias, normalize; DVE: recips, AT/y evictions.

GROUPS_V4 = [[0, 1, 2, 3], [4, 5, 6, 7]]


def build_mha_v4(reps=1, debug=False):
    nc = bass.Bass(num_devices=8)
    xTq_d = nc.declare_dram_parameter("xTq", [D, SQ], BF16, isOutput=False)
    WqT_d = nc.declare_dram_parameter("WqT", [D, D], BF16, isOutput=False)
    WkT_d = nc.declare_dram_parameter("WkT", [D, D], BF16, isOutput=False)
    WvT_d = nc.declare_dram_parameter("WvT", [D, D], BF16, isOutput=False)
    WoT_d = nc.declare_dram_parameter("WoT", [D, D], BF16, isOutput=False)
    bqt_d = nc.declare_dram_parameter("bqt", [128, NT_D], F32, isOutput=False)
    bkt_d = nc.declare_dram_parameter("bkt", [128, NT_D], F32, isOutput=False)
    bvr_d = nc.declare_dram_parameter("bvr", [1, D], BF16, isOutput=False)
    bor_d = nc.declare_dram_parameter("bor", [1, D], BF16, isOutput=False)
    ident_d = nc.declare_dram_parameter("ident", [128, 128], F32, isOutput=False)
    Y_d = nc.declare_dram_parameter("y", [SQ, D], F32, isOutput=True)
    if debug:
        KT_dbg = nc.declare_dram_parameter("kt_dbg", [128, NT_D, S], BF16, isOutput=True)
        VS_dbg = nc.declare_dram_parameter(
            "vs_dbg", [128, NT_S, H, DK + 1], BF16, isOutput=True
        )
        QT_dbg = nc.declare_dram_parameter("qt_dbg", [128, NT_D, SQ], BF16, isOutput=True)
        AT_dbg = nc.declare_dram_parameter("at_dbg", [128, NT_D, SQ], BF16, isOutput=True)

    wkT_r = WkT_d.rearrange("(c p) o -> p c o", p=128)
    wqT_r = WqT_d.rearrange("(c p) o -> p c o", p=128)
    NCH = 4  # chunks per batch group

    with tile.TileContext(nc) as tc:
        with (
            tc.tile_pool(name="persist", bufs=1) as persist,
            tc.tile_pool(name="stage", bufs=1) as stage,
            tc.tile_pool(name="wk", bufs=2) as wkp,
            tc.tile_pool(name="wq", bufs=2) as wqp,
            tc.tile_pool(name="es", bufs=3) as esp,
            tc.tile_pool(name="au", bufs=2) as aup,
            tc.tile_pool(name="small", bufs=2) as smallp,
            tc.tile_pool(name="ys", bufs=2) as ysp,
            tc.tile_pool(name="dram", bufs=1, space="DRAM") as dram,
            tc.tile_pool(name="ps_a", bufs=2, space="PSUM") as ps_a,
            tc.tile_pool(name="ps_qk", bufs=2, space="PSUM") as ps_qk,
            tc.tile_pool(name="ps_pv", bufs=2, space="PSUM") as ps_pv,
        ):
            KT = persist.tile([128, NT_D, S], BF16, name="KT")
            VS = persist.tile([128, NT_S, H, DK + 1], BF16, name="VS")
            QT = persist.tile([128, NT_D, SQ], BF16, name="QT")
            AT = persist.tile([128, NT_D, SQ], BF16, name="AT")
            bqt = persist.tile([128, NT_D], F32, name="bqt")
            bkt = persist.tile([128, NT_D], F32, name="bkt")
            bvb = persist.tile([128, D], BF16, name="bvb")
            bob = persist.tile([128, D], BF16, name="bob")
            ident = persist.tile([128, 128], F32, name="ident")

            xTq = stage.tile([128, NT_D, SQ], BF16, name="xTq")
            WvT = stage.tile([128, NT_D, D], BF16, name="WvT")
            WoT = stage.tile([128, NT_D, D], BF16, name="WoT")
            ko = stage.tile([128, NT_D, SQ], BF16, name="ko")
            vo = stage.tile([128, NT_S // NCH, H, DK + 1], BF16, name="vo")

            kk_in_a = dram.tile([128, NT_D // 2, SQ], BF16, name="kk_in_a")
            kk_in_b = dram.tile([128, NT_D // 2, SQ], BF16, name="kk_in_b")
            kk_out_a = dram.tile([NCH, 128, NT_D // 2, SQ], BF16, name="kk_out_a")
            kk_out_b = dram.tile([NCH, 128, NT_D // 2, SQ], BF16, name="kk_out_b")
            vv_in = dram.tile([128, NT_S // NCH, H, DK + 1], BF16, name="vv_in")
            vv_out = dram.tile(
                [NCH, 128, NT_S // NCH, H, DK + 1], BF16, name="vv_out"
            )

            def body():
                nc.sync.dma_start(out=xTq[:], in_=xTq_d.rearrange("(c p) s -> p c s", p=128))
                nc.sync.dma_start(out=bqt[:], in_=bqt_d[:])
                nc.sync.dma_start(out=bkt[:], in_=bkt_d[:])
                nc.sync.dma_start(out=bvb[:], in_=bvr_d.broadcast_to([128, D]))
                nc.sync.dma_start(out=bob[:], in_=bor_d.broadcast_to([128, D]))
                nc.sync.dma_start(out=ident[:], in_=ident_d[:])
                for dt in range(NT_D):
                    nc.sync.dma_start(
                        out=WvT[:, dt, :],
                        in_=WvT_d.rearrange("(c p) o -> p c o", p=128)[:, dt, :],
                    )

                # Phase A order: K(ot0-3) -> CC-K1 -> V -> CC-V -> Q(ot0-3)
                # -> K(ot4-7) -> CC-K2 -> Q(ot4-7).  Heads 0-7 only need the
                # first K half + first Q half, so attention (and the ACT exp
                # stream, the kernel's critical path) starts as soon as the
                # 2MB half-gather lands instead of waiting for all of K.  All
                # phase-A evictions run on DVE so ACT stays Exp-only (no
                # activation-table reloads mid-stream).

                def k_proj(ot):
                    wk = wkp.tile([128, NT_D, 128], BF16, name="wk")
                    nc.sync.dma_start(out=wk[:], in_=wkT_r[:, :, ot * 128 : (ot + 1) * 128])
                    pk = ps_a.tile([128, 512], F32, name="pa")
                    for dt in range(NT_D):
                        nc.tensor.matmul(
                            pk[:],
                            wk[:, dt, :],
                            xTq[:, dt, :],
                            start=(dt == 0),
                            stop=(dt == NT_D - 1),
                        )
                    nc.vector.tensor_scalar_add(ko[:, ot, :], pk[:], bkt[:, ot : ot + 1])
                    kk_in = kk_in_a if ot < 4 else kk_in_b
                    nc.sync.dma_start(out=kk_in[:, ot % 4, :], in_=ko[:, ot, :])

                def q_proj(ot):
                    wq = wqp.tile([128, NT_D, 128], BF16, name="wq")
                    nc.sync.dma_start(out=wq[:], in_=wqT_r[:, :, ot * 128 : (ot + 1) * 128])
                    pq = ps_a.tile([128, 512], F32, name="pa")
                    for dt in range(NT_D):
                        nc.tensor.matmul(
                            pq[:],
                            wq[:, dt, :],
                            xTq[:, dt, :],
                            start=(dt == 0),
                            stop=(dt == NT_D - 1),
                        )
                    nc.vector.tensor_scalar_add(QT[:, ot, :], pq[:], bqt[:, ot : ot + 1])

                def cc_k(half):
                    kk_in = kk_in_a if half == 0 else kk_in_b
                    kk_out = kk_out_a if half == 0 else kk_out_b
                    nc.gpsimd.collective_compute(
                        "AllGather",
                        mybir.AluOpType.bypass,
                        replica_groups=GROUPS_V4,
                        ins=[kk_in.opt()],
                        outs=[kk_out.opt()],
                    )
                    for g in range(NCH):
                        nc.gpsimd.dma_start(
                            out=KT[:, 4 * half : 4 * half + 4, g * SQ : (g + 1) * SQ],
                            in_=kk_out[g],
                        )

                for ot in range(4):
                    k_proj(ot)
                cc_k(0)

                # ---- V projection (own chunk) + bounce ----
                nc.vector.memset(vo[:, :, :, DK : DK + 1], 1.0)
                for st in range(NT_S // NCH):
                    for oc in range(2):
                        pv = ps_a.tile([128, 512], F32, name="pa")
                        for dt in range(NT_D):
                            nc.tensor.matmul(
                                pv[:],
                                xTq[:, dt, st * 128 : (st + 1) * 128],
                                WvT[:, dt, oc * 512 : (oc + 1) * 512],
                                start=(dt == 0),
                                stop=(dt == NT_D - 1),
                            )
                        nc.vector.tensor_tensor(
                            out=vo[:, st, oc * 8 : (oc + 1) * 8, 0:DK],
                            in0=pv[:].rearrange("p (h d) -> p h d", d=DK),
                            in1=bvb[:, oc * 512 : (oc + 1) * 512].rearrange(
                                "p (h d) -> p h d", d=DK
                            ),
                            op=mybir.AluOpType.add,
                        )
                    nc.sync.dma_start(out=vv_in[:, st], in_=vo[:, st])
                nc.gpsimd.collective_compute(
                    "AllGather",
                    mybir.AluOpType.bypass,
                    replica_groups=GROUPS_V4,
                    ins=[vv_in.opt()],
                    outs=[vv_out.opt()],
                )
                for g in range(NCH):
                    nc.gpsimd.dma_start(
                        out=VS[:, g * 4 : (g + 1) * 4], in_=vv_out[g]
                    )
                nc.sync.dma_start(out=WoT[:], in_=WoT_d.rearrange("(c p) o -> p c o", p=128))

                for ot in range(4):
                    q_proj(ot)
                for ot in range(4, NT_D):
                    k_proj(ot)
                cc_k(1)
                for ot in range(4, NT_D):
                    q_proj(ot)

                # ---- attention ----
                # Per head: 8 QK/exp units fill a whole-head es tile; PV then
                # runs one accumulation group per q-tile (PSUM allows a single
                # pending group per 2KB bank), interleaved into the NEXT
                # head's QK stream so ACT never starves.
                NSG = NT_S // 2  # 8 sg units per head
                es_tiles = {}
                au_tiles = {}

                def emit_qk(h, sg):
                    ot, po = h // 2, DK * (h % 2)
                    pqk = ps_qk.tile([128, 2, 512], F32, name="pqk")
                    for j in range(2):
                        st = 2 * sg + j
                        nc.tensor.matmul(
                            pqk[:, j, :],
                            KT[po : po + DK, ot, st * 128 : (st + 1) * 128],
                            QT[po : po + DK, ot, :],
                            start=True,
                            stop=True,
                        )
                    if sg == 0:
                        es_tiles[h] = esp.tile([128, NT_S, 512], BF16, name="es")
                    nc.scalar.activation(
                        es_tiles[h][:, 2 * sg : 2 * sg + 2, :].rearrange(
                            "p a b -> p (a b)"
                        ),
                        pqk[:].rearrange("p a b -> p (a b)"),
                        AF.Exp,
                        scale=0.125,
                    )

                def emit_pv(h, qt):
                    es = es_tiles[h]
                    ppv = ps_pv.tile([128, DK + 1], F32, name="ppv")
                    for st in range(NT_S):
                        nc.tensor.matmul(
                            ppv[:],
                            es[:, st, qt * 128 : (qt + 1) * 128],
                            VS[:, st, h, :],
                            start=(st == 0),
                            stop=(st == NT_S - 1),
                        )
                    if qt == NT_Q - 1:
                        es_tiles.pop(h)
                    r1 = smallp.tile([128, 1], F32, name="rQ")
                    nc.vector.reciprocal(out=r1[:], in_=ppv[:, DK : DK + 1])
                    if h % 2 == 0 and qt == 0:
                        au_tiles[h // 2] = aup.tile([128, NT_Q, 128], F32, name="au")
                    au = au_tiles[h // 2]
                    nc.vector.tensor_scalar(
                        out=au[:, qt, (h % 2) * DK : (h % 2) * DK + DK],
                        in0=ppv[:, 0:DK],
                        scalar1=r1[:],
                        scalar2=None,
                        op0=mybir.AluOpType.mult,
                    )

                def emit_transposes(ot):
                    au_done = au_tiles.pop(ot)
                    for qt in range(NT_Q):
                        pt = ps_a.tile([128, 128], F32, name="pa")
                        nc.tensor.transpose(pt[:], au_done[:, qt, :], ident[:])
                        nc.vector.tensor_copy(
                            out=AT[:, ot, qt * 128 : (qt + 1) * 128], in_=pt[:]
                        )

                # deferred-work queue: after qk unit k of head h, run pending
                # PV q-tile groups of head h-1 (slots after units 1..4) and
                # transposes of the completed pair (slot after unit 5).
                for h in range(H):
                    for sg in range(NSG):
                        emit_qk(h, sg)
                        if h >= 1 and 1 <= sg <= NT_Q:
                            emit_pv(h - 1, sg - 1)
                        if h >= 2 and h % 2 == 0 and sg == NT_Q + 1:
                            emit_transposes(h // 2 - 1)
                for qt in range(NT_Q):
                    emit_pv(H - 1, qt)
                emit_transposes(H // 2 - 1)

                # ---- output projection ----
                for qt in range(NT_Q):
                    for oc in range(2):
                        py = ps_pv.tile([128, 512], F32, name="ppv")
                        for ct in range(NT_D):
                            nc.tensor.matmul(
                                py[:],
                                AT[:, ct, qt * 128 : (qt + 1) * 128],
                                WoT[:, ct, oc * 512 : (oc + 1) * 512],
                                start=(ct == 0),
                                stop=(ct == NT_D - 1),
                            )
                        ys = ysp.tile([128, 512], F32, name="ys")
                        nc.vector.tensor_tensor(
                            out=ys[:],
                            in0=py[:],
                            in1=bob[:, oc * 512 : (oc + 1) * 512],
                            op=mybir.AluOpType.add,
                        )
                        nc.sync.dma_start(
                            out=Y_d[qt * 128 : (qt + 1) * 128, oc * 512 : (oc + 1) * 512],
                            in_=ys[:],
                        )
                if debug:
                    nc.sync.dma_start(out=KT_dbg[:], in_=KT[:])
                    nc.sync.dma_start(out=VS_dbg[:], in_=VS[:])
                    nc.sync.dma_start(out=QT_dbg[:], in_=QT[:])
                    nc.sync.dma_start(out=AT_dbg[:], in_=AT[:])

            # collectives are not usable inside For_i (mesh desync), so the
            # timing variant unrolls the body instead
            for _ in range(reps):
                body()

    _split_sync_waits(nc)
    return nc


def prep_core_inputs_v4(x, Wq, bq, Wk, bk, Wv, bv, Wo, bo):
    bf = ml_dtypes.bfloat16
    x = np.asarray(x, dtype=np.float32)
    WqT = np.ascontiguousarray(np.asarray(Wq).T).astype(bf)
    WkT = np.ascontiguousarray(np.asarray(Wk).T).astype(bf)
    WvT = np.ascontiguousarray(np.asarray(Wv).T).astype(bf)
    WoT = np.ascontiguousarray(np.asarray(Wo).T).astype(bf)
    bqt = np.ascontiguousarray(np.asarray(bq, dtype=np.float32).reshape(NT_D, 128).T)
    bkt = np.ascontiguousarray(np.asarray(bk, dtype=np.float32).reshape(NT_D, 128).T)
    bvr = np.asarray(bv).reshape(1, D).astype(bf)
    bor = np.asarray(bo).reshape(1, D).astype(bf)
    ident = np.eye(128, dtype=np.float32)
    in_maps = []
    for c in range(N_CORES):
        b, qc = c // 4, c % 4
        xTq = np.ascontiguousarray(x[b, qc * SQ : (qc + 1) * SQ].T).astype(bf)
        in_maps.append(
            dict(
                xTq=xTq, WqT=WqT, WkT=WkT, WvT=WvT, WoT=WoT,
                bqt=bqt, bkt=bkt, bvr=bvr, bor=bor, ident=ident,
            )
        )
    return in_maps


# ---------------------------------------------------------------------------
# Kernel builder
# ---------------------------------------------------------------------------


def build_mha_v2(reps=1, salt=0):
    """Interleaved variant: V projection first, then per-o-tile K/Q projection
    immediately followed by the two heads that consume that o-tile, so
    projection and attention overlap on all engines.  All pools co-resident
    (23 MB SBUF, 8 PSUM banks exactly); Wk/Wq streamed per o-tile.  V and y
    biases are added on DVE during PSUM eviction against DMA-broadcast bias
    tiles instead of K=1 matmuls.

    reps > 1 wraps the body in an on-device For_i loop (timing variant).
    """
    nc = bass.Bass()
    xT_d = nc.declare_dram_parameter("xT", [D, S], BF16, isOutput=False)
    xTq_d = nc.declare_dram_parameter("xTq", [D, SQ], BF16, isOutput=False)
    WqT_d = nc.declare_dram_parameter("WqT", [D, D], BF16, isOutput=False)
    WkT_d = nc.declare_dram_parameter("WkT", [D, D], BF16, isOutput=False)
    WvT_d = nc.declare_dram_parameter("WvT", [D, D], BF16, isOutput=False)
    WoT_d = nc.declare_dram_parameter("WoT", [D, D], BF16, isOutput=False)
    bqt_d = nc.declare_dram_parameter("bqt", [128, NT_D], F32, isOutput=False)
    bkt_d = nc.declare_dram_parameter("bkt", [128, NT_D], F32, isOutput=False)
    bvr_d = nc.declare_dram_parameter("bvr", [1, D], BF16, isOutput=False)
    bor_d = nc.declare_dram_parameter("bor", [1, D], BF16, isOutput=False)
    Y_d = nc.declare_dram_parameter("y", [SQ, D], F32, isOutput=True)

    wkT_r = WkT_d.rearrange("(c p) o -> p c o", p=128)
    wqT_r = WqT_d.rearrange("(c p) o -> p c o", p=128)

    with tile.TileContext(nc) as tc:
        with (
            tc.tile_pool(name="persist", bufs=1) as persist,
            tc.tile_pool(name="ph1", bufs=1) as ph1,
            tc.tile_pool(name="wk", bufs=2) as wkp,
            tc.tile_pool(name="wq", bufs=2) as wqp,
            tc.tile_pool(name="es", bufs=3) as esp,
            tc.tile_pool(name="small", bufs=2) as small,
            tc.tile_pool(name="ph3", bufs=1) as ph3,
            tc.tile_pool(name="ps_big", bufs=2, space="PSUM") as ps_big,
            tc.tile_pool(name="ps_mm", bufs=2, space="PSUM") as ps_mm,
            tc.tile_pool(name="ps_pv", bufs=2, space="PSUM") as ps_pv,
        ):
            KT = persist.tile([128, NT_D, S], BF16, name="KT")
            QT = persist.tile([128, NT_D, SQ], BF16, name="QT")
            VS = persist.tile([128, NT_S, H, DK + 1], BF16, name="VS")
            AT = persist.tile([128, NT_D, SQ], BF16, name="AT")
            bqt = persist.tile([128, NT_D], F32, name="bqt")
            bkt = persist.tile([128, NT_D], F32, name="bkt")
            bvb = persist.tile([128, D], BF16, name="bvb")
            bob = persist.tile([128, D], BF16, name="bob")
            ones64r = persist.tile([1, DK], F32R, name="ones64r")
            ones64f = persist.tile([1, DK], F32, name="ones64f")

            xT = ph1.tile([128, NT_D, S], BF16, name="xT")
            xTq = ph1.tile([128, NT_D, SQ], BF16, name="xTq")
            WvT = ph1.tile([128, NT_D, D], BF16, name="WvT")
            WoT = ph3.tile([128, NT_D, D], BF16, name="WoT")

            def body():
                # V operands first -- V production is the kernel's opening act
                for dt in range(NT_D):
                    nc.sync.dma_start(
                        out=WvT[:, dt, :],
                        in_=WvT_d.rearrange("(c p) o -> p c o", p=128)[:, dt, :],
                    )
                for sc in range(4):
                    nc.sync.dma_start(
                        out=xT[:, :, sc * 512 : (sc + 1) * 512],
                        in_=xT_d.rearrange("(c p) s -> p c s", p=128)[
                            :, :, sc * 512 : (sc + 1) * 512
                        ],
                    )
                nc.sync.dma_start(out=xTq[:], in_=xTq_d.rearrange("(c p) s -> p c s", p=128))
                nc.sync.dma_start(out=bqt[:], in_=bqt_d[:])
                nc.sync.dma_start(out=bkt[:], in_=bkt_d[:])
                # partition-broadcast bias rows (DRAM source, partition step 0)
                nc.sync.dma_start(out=bvb[:], in_=bvr_d.broadcast_to([128, D]))
                nc.sync.dma_start(out=bob[:], in_=bor_d.broadcast_to([128, D]))
                nc.vector.memset(ones64f[:], 1.0)
                nc.vector.tensor_copy(out=ones64r[:], in_=ones64f[:])
                nc.vector.memset(VS[:, :, :, DK : DK + 1], 1.0)
                nc.sync.dma_start(out=WoT[:], in_=WoT_d.rearrange("(c p) o -> p c o", p=128))

                # ---- V production ----
                for st in range(NT_S):
                    for oc in range(2):
                        pv = ps_mm.tile([128, 512], F32, name="mm")
                        for dt in range(NT_D):
                            nc.tensor.matmul(
                                pv[:],
                                xT[:, dt, st * 128 : (st + 1) * 128],
                                WvT[:, dt, oc * 512 : (oc + 1) * 512],
                                start=(dt == 0),
                                stop=(dt == NT_D - 1),
                            )
                        nc.vector.tensor_tensor(
                            out=VS[:, st, oc * 8 : (oc + 1) * 8, 0:DK],
                            in0=pv[:].rearrange("p (h d) -> p h d", d=DK),
                            in1=bvb[:, oc * 512 : (oc + 1) * 512].rearrange(
                                "p (h d) -> p h d", d=DK
                            ),
                            op=mybir.AluOpType.add,
                        )

                # ---- per-o-tile projection + the two heads that consume it ----
                for ot in range(NT_D):
                    wk = wkp.tile([128, NT_D, 128], BF16, name="wk")
                    nc.sync.dma_start(out=wk[:], in_=wkT_r[:, :, ot * 128 : (ot + 1) * 128])
                    for half in range(2):
                        pk2 = ps_big.tile([128, 2, 512], F32, name="big2")
                        for sc in range(2):
                            for dt in range(NT_D):
                                nc.tensor.matmul(
                                    pk2[:, sc, :],
                                    wk[:, dt, :],
                                    xT[:, dt, (2 * half + sc) * 512 : (2 * half + sc + 1) * 512],
                                    start=(dt == 0),
                                    stop=(dt == NT_D - 1),
                                )
                        nc.scalar.activation(
                            KT[:, ot, half * 1024 : (half + 1) * 1024],
                            pk2[:, :, :].rearrange("p a b -> p (a b)"),
                            AF.Identity,
                            bias=bkt[:, ot : ot + 1],
                        )
                    wq = wqp.tile([128, NT_D, 128], BF16, name="wq")
                    nc.sync.dma_start(out=wq[:], in_=wqT_r[:, :, ot * 128 : (ot + 1) * 128])
                    pq = ps_mm.tile([128, 512], F32, name="mm")
                    for dt in range(NT_D):
                        nc.tensor.matmul(
                            pq[:],
                            wq[:, dt, :],
                            xTq[:, dt, :],
                            start=(dt == 0),
                            stop=(dt == NT_D - 1),
                        )
                    nc.scalar.activation(
                        QT[:, ot, :], pq[:], AF.Identity, bias=bqt[:, ot : ot + 1]
                    )

                    for h in (2 * ot, 2 * ot + 1):
                        po = DK * (h % 2)
                        ppv = ps_pv.tile([DK + 1, 512], F32, name="ppv")
                        for half in range(2):
                            es = esp.tile([128, NT_S // 2, 512], BF16, name="es")
                            for sg in range(4):
                                pqk = ps_big.tile([128, 2, 512], F32, name="big2")
                                for j in range(2):
                                    st = half * 8 + 2 * sg + j
                                    nc.tensor.matmul(
                                        pqk[:, j, :],
                                        KT[po : po + DK, ot, st * 128 : (st + 1) * 128],
                                        QT[po : po + DK, ot, :],
                                        start=True,
                                        stop=True,
                                    )
                                nc.scalar.activation(
                                    es[:, 2 * sg : 2 * sg + 2, :].rearrange(
                                        "p a b -> p (a b)"
                                    ),
                                    pqk[:].rearrange("p a b -> p (a b)"),
                                    AF.Exp,
                                    scale=0.125,
                                )
                            for j in range(NT_S // 2):
                                st = half * 8 + j
                                nc.tensor.matmul(
                                    ppv[:],
                                    VS[:, st, h, :],
                                    es[:, j, :],
                                    start=(st == 0),
                                    stop=(st == NT_S - 1),
                                )
                        r = small.tile([1, 512], F32R, name="r")
                        with nc.allow_low_precision(reason="f32r recip, 1.2e-4 rel"):
                            nc.vector.reciprocal(out=r[:], in_=ppv[DK : DK + 1, :])
                        prb = ps_mm.tile([DK, 512], F32, name="mm")
                        nc.tensor.matmul(prb[:], ones64r[:], r[:], start=True, stop=True)
                        au = small.tile([DK, 512], F32, name="au")
                        nc.vector.tensor_copy(out=au[:], in_=ppv[0:DK, :])
                        nc.vector.tensor_mul(
                            out=AT[po : po + DK, ot, :], in0=au[:], in1=prb[:]
                        )

                # ---- output projection ----
                for qt in range(NT_Q):
                    for oc in range(2):
                        py = ps_big.tile([128, 512], F32, name="big2")
                        for ct in range(NT_D):
                            nc.tensor.matmul(
                                py[:],
                                AT[:, ct, qt * 128 : (qt + 1) * 128],
                                WoT[:, ct, oc * 512 : (oc + 1) * 512],
                                start=(ct == 0),
                                stop=(ct == NT_D - 1),
                            )
                        ys = small.tile([128, 512], F32, name="au")
                        nc.vector.tensor_tensor(
                            out=ys[:],
                            in0=py[:],
                            in1=bob[:, oc * 512 : (oc + 1) * 512],
                            op=mybir.AluOpType.add,
                        )
                        nc.sync.dma_start(
                            out=Y_d[qt * 128 : (qt + 1) * 128, oc * 512 : (oc + 1) * 512],
                            in_=ys[:],
                        )


            if reps > 1:
                with tc.For_i(0, reps, 1):
                    body()
            else:
                body()

    _split_sync_waits(nc)
    return nc


def build_mha(reps=1):
    """reps > 1 wraps the body in an on-device For_i loop (timing variant)."""
    nc = bass.Bass()
    xT_d = nc.declare_dram_parameter("xT", [D, S], BF16, isOutput=False)
    xTq_d = nc.declare_dram_parameter("xTq", [D, SQ], BF16, isOutput=False)
    WqT_d = nc.declare_dram_parameter("WqT", [D, D], BF16, isOutput=False)
    WkT_d = nc.declare_dram_parameter("WkT", [D, D], BF16, isOutput=False)
    WvT_d = nc.declare_dram_parameter("WvT", [D, D], BF16, isOutput=False)
    WoT_d = nc.declare_dram_parameter("WoT", [D, D], BF16, isOutput=False)
    bqt_d = nc.declare_dram_parameter("bqt", [128, NT_D], F32, isOutput=False)
    bkt_d = nc.declare_dram_parameter("bkt", [128, NT_D], F32, isOutput=False)
    bvr_d = nc.declare_dram_parameter("bvr", [1, D], BF16, isOutput=False)
    bor_d = nc.declare_dram_parameter("bor", [1, D], BF16, isOutput=False)
    Y_d = nc.declare_dram_parameter("y", [SQ, D], F32, isOutput=True)

    with tile.TileContext(nc) as tc:
        with tc.tile_pool(name="persist", bufs=1) as persist:
            KT = persist.tile([128, NT_D, S], BF16, name="KT")
            QT = persist.tile([128, NT_D, SQ], BF16, name="QT")
            VS = persist.tile([128, NT_S, H, DK + 1], BF16, name="VS")
            AT = persist.tile([128, NT_D, SQ], BF16, name="AT")
            bqt = persist.tile([128, NT_D], F32, name="bqt")
            bkt = persist.tile([128, NT_D], F32, name="bkt")
            bvr = persist.tile([1, D], BF16, name="bvr")
            bor = persist.tile([1, D], BF16, name="bor")
            ones1 = persist.tile([1, 128], BF16, name="ones1")
            ones64r = persist.tile([1, DK], F32R, name="ones64r")
            ones64f = persist.tile([1, DK], F32, name="ones64f")

            nc.sync.dma_start(out=bqt[:], in_=bqt_d[:])
            nc.sync.dma_start(out=bkt[:], in_=bkt_d[:])
            nc.sync.dma_start(out=bvr[:], in_=bvr_d[:])
            nc.sync.dma_start(out=bor[:], in_=bor_d[:])
            nc.vector.memset(ones1[:], 1.0)
            nc.vector.memset(ones64f[:], 1.0)
            nc.vector.tensor_copy(out=ones64r[:], in_=ones64f[:])
            nc.vector.memset(VS[:, :, :, DK : DK + 1], 1.0)

            def phase1():
                with (
                    tc.tile_pool(name="ph1", bufs=1) as ph1,
                    tc.tile_pool(name="ps_kt", bufs=1, space="PSUM") as ps_kt,
                    tc.tile_pool(name="ps_mm", bufs=2, space="PSUM") as ps_mm,
                ):
                    xT = ph1.tile([128, NT_D, S], BF16, name="xT")
                    xTq = ph1.tile([128, NT_D, SQ], BF16, name="xTq")
                    WkT = ph1.tile([128, NT_D, D], BF16, name="WkT")
                    WqT = ph1.tile([128, NT_D, D], BF16, name="WqT")
                    WvT = ph1.tile([128, NT_D, D], BF16, name="WvT")
                    nc.sync.dma_start(
                        out=xT[:], in_=xT_d.rearrange("(c p) s -> p c s", p=128)
                    )
                    nc.sync.dma_start(
                        out=WkT[:], in_=WkT_d.rearrange("(c p) o -> p c o", p=128)
                    )
                    nc.sync.dma_start(
                        out=xTq[:], in_=xTq_d.rearrange("(c p) s -> p c s", p=128)
                    )
                    nc.sync.dma_start(
                        out=WqT[:], in_=WqT_d.rearrange("(c p) o -> p c o", p=128)
                    )
                    nc.sync.dma_start(
                        out=WvT[:], in_=WvT_d.rearrange("(c p) o -> p c o", p=128)
                    )

                    for ot in range(NT_D):
                        # K.T block [o-tile, all s] in a 4-bank psum tile
                        pk = ps_kt.tile([128, 4, 512], F32, name="pk")
                        for sc in range(4):
                            for dt in range(NT_D):
                                nc.tensor.matmul(
                                    pk[:, sc, :],
                                    WkT[:, dt, ot * 128 : (ot + 1) * 128],
                                    xT[:, dt, sc * 512 : (sc + 1) * 512],
                                    start=(dt == 0),
                                    stop=(dt == NT_D - 1),
                                )
                        nc.scalar.activation(
                            KT[:, ot, :],
                            pk[:, :, :].rearrange("p a b -> p (a b)"),
                            AF.Identity,
                            bias=bkt[:, ot : ot + 1],
                        )

                        # V for two s-tiles (fills PE while ACT evicts K.T)
                        for st in (2 * ot, 2 * ot + 1):
                            for oc in range(2):
                                pv = ps_mm.tile([128, 512], F32, name="mm")
                                for dt in range(NT_D):
                                    nc.tensor.matmul(
                                        pv[:],
                                        xT[:, dt, st * 128 : (st + 1) * 128],
                                        WvT[:, dt, oc * 512 : (oc + 1) * 512],
                                        start=(dt == 0),
                                        stop=False,
                                    )
                                nc.tensor.matmul(
                                    pv[:],
                                    ones1[:],
                                    bvr[:, oc * 512 : (oc + 1) * 512],
                                    start=False,
                                    stop=True,
                                )
                                nc.vector.tensor_copy(
                                    out=VS[:, st, oc * 8 : (oc + 1) * 8, 0:DK],
                                    in_=pv[:].rearrange("p (h d) -> p h d", d=DK),
                                )

                        # Q.T block
                        pq = ps_mm.tile([128, 512], F32, name="mm")
                        for dt in range(NT_D):
                            nc.tensor.matmul(
                                pq[:],
                                WqT[:, dt, ot * 128 : (ot + 1) * 128],
                                xTq[:, dt, :],
                                start=(dt == 0),
                                stop=(dt == NT_D - 1),
                            )
                        nc.scalar.activation(
                            QT[:, ot, :], pq[:], AF.Identity, bias=bqt[:, ot : ot + 1]
                        )

            def phase23():
                with (
                    tc.tile_pool(name="ph2", bufs=1) as ph2,
                    tc.tile_pool(name="es", bufs=3) as esp,
                    tc.tile_pool(name="small", bufs=2) as small,
                    tc.tile_pool(name="ps_qk", bufs=2, space="PSUM") as ps_qk,
                    tc.tile_pool(name="ps_pv", bufs=2, space="PSUM") as ps_pv,
                    tc.tile_pool(name="ps_rb", bufs=1, space="PSUM") as ps_rb,
                ):
                    WoT = ph2.tile([128, NT_D, D], BF16, name="WoT")
                    nc.sync.dma_start(
                        out=WoT[:], in_=WoT_d.rearrange("(c p) o -> p c o", p=128)
                    )

                    for h in range(H):
                        ot, po = h // 2, DK * (h % 2)
                        es = esp.tile([128, NT_S, 512], BF16, name="es")
                        for sg in range(NT_S // 2):
                            pqk = ps_qk.tile([128, 2, 512], F32, name="pqk")
                            for j in range(2):
                                st = 2 * sg + j
                                nc.tensor.matmul(
                                    pqk[:, j, :],
                                    KT[po : po + DK, ot, st * 128 : (st + 1) * 128],
                                    QT[po : po + DK, ot, :],
                                    start=True,
                                    stop=True,
                                )
                            nc.scalar.activation(
                                es[:, 2 * sg : 2 * sg + 2, :].rearrange(
                                    "p a b -> p (a b)"
                                ),
                                pqk[:].rearrange("p a b -> p (a b)"),
                                AF.Exp,
                                scale=0.125,
                            )
                        ppv = ps_pv.tile([DK + 1, 512], F32, name="ppv")
                        for st in range(NT_S):
                            nc.tensor.matmul(
                                ppv[:],
                                VS[:, st, h, :],
                                es[:, st, :],
                                start=(st == 0),
                                stop=(st == NT_S - 1),
                            )
                        r = small.tile([1, 512], F32R, name="r")
                        with nc.allow_low_precision(reason="f32r recip, 1.2e-4 rel"):
                            nc.vector.reciprocal(out=r[:], in_=ppv[DK : DK + 1, :])
                        prb = ps_rb.tile([DK, 512], F32, name="prb")
                        nc.tensor.matmul(prb[:], ones64r[:], r[:], start=True, stop=True)
                        au = small.tile([DK, 512], F32, name="au")
                        nc.vector.tensor_copy(out=au[:], in_=ppv[0:DK, :])
                        nc.vector.tensor_mul(
                            out=AT[po : po + DK, ot, :], in0=au[:], in1=prb[:]
                        )

                    # ---- phase 3: output projection ----
                    for qt in range(NT_Q):
                        for oc in range(2):
                            py = ps_pv.tile([128, 512], F32, name="ppv")
                            for ct in range(NT_D):
                                nc.tensor.matmul(
                                    py[:],
                                    AT[:, ct, qt * 128 : (qt + 1) * 128],
                                    WoT[:, ct, oc * 512 : (oc + 1) * 512],
                                    start=(ct == 0),
                                    stop=False,
                                )
                            nc.tensor.matmul(
                                py[:],
                                ones1[:],
                                bor[:, oc * 512 : (oc + 1) * 512],
                                start=False,
                                stop=True,
                            )
                            ys = small.tile([128, 512], F32, name="ys")
                            nc.vector.tensor_copy(out=ys[:], in_=py[:])
                            nc.sync.dma_start(
                                out=Y_d[
                                    qt * 128 : (qt + 1) * 128,
                                    oc * 512 : (oc + 1) * 512,
                                ],
                                in_=ys[:],
                            )

            if reps > 1:
                with tc.For_i(0, reps, 1):
                    phase1()
                    phase23()
            else:
                phase1()
                phase23()

    _split_sync_waits(nc)
    return nc


# ---------------------------------------------------------------------------
# Host-side sharding / unsharding
# ---------------------------------------------------------------------------


def prep_core_inputs(x, Wq, bq, Wk, bk, Wv, bv, Wo, bo):
    bf = ml_dtypes.bfloat16
    x = np.asarray(x, dtype=np.float32)
    WqT = np.ascontiguousarray(np.asarray(Wq).T).astype(bf)
    WkT = np.ascontiguousarray(np.asarray(Wk).T).astype(bf)
    WvT = np.ascontiguousarray(np.asarray(Wv).T).astype(bf)
    WoT = np.ascontiguousarray(np.asarray(Wo).T).astype(bf)
    bqt = np.ascontiguousarray(np.asarray(bq, dtype=np.float32).reshape(NT_D, 128).T)
    bkt = np.ascontiguousarray(np.asarray(bk, dtype=np.float32).reshape(NT_D, 128).T)
    bvr = np.asarray(bv).reshape(1, D).astype(bf)
    bor = np.asarray(bo).reshape(1, D).astype(bf)
    in_maps = []
    for c in range(N_CORES):
        b, qc = c // 4, c % 4
        xb = x[b]
        xT = np.ascontiguousarray(xb.T).astype(bf)
        xTq = np.ascontiguousarray(xb[qc * SQ : (qc + 1) * SQ].T).astype(bf)
        in_maps.append(
            dict(
                xT=xT, xTq=xTq, WqT=WqT, WkT=WkT, WvT=WvT, WoT=WoT,
                bqt=bqt, bkt=bkt, bvr=bvr, bor=bor,
            )
        )
    return in_maps


def assemble_output(outs):
    y = np.empty((2, S, D), dtype=np.float32)
    for c in range(N_CORES):
        b, qc = c // 4, c % 4
        y[b, qc * SQ : (qc + 1) * SQ, :] = outs[c]["y"]
    return y


_NC_CACHE = {}


def kernel(**inputs) -> np.ndarray:
    import time

    from concourse.bass_utils import run_bass_kernel_spmd

    if "nc" not in _NC_CACHE:
        _NC_CACHE["nc"] = build_mha_v4()
    nc = _NC_CACHE["nc"]
    in_maps = prep_core_inputs_v4(**inputs)
    # The tunnel-attached device occasionally reports
    # NRT_EXEC_UNIT_UNRECOVERABLE right after a prior heavy run; it recovers
    # on its own within ~90 s.  Retry once before giving up.
    try:
        res = run_bass_kernel_spmd(nc, in_maps, core_ids=list(range(N_CORES)))
    except Exception:
        time.sleep(90)
        res = run_bass_kernel_spmd(nc, in_maps, core_ids=list(range(N_CORES)))
    return assemble_output(res.results)



# revision 24
# speedup vs baseline: 1.2459x; 1.2459x over previous
"""Multi-head attention (B=2, S=2048, D=1024, H=16) on 8 TRN2 NeuronCores.

Active variant (build_mha_v4): sequence-data-parallel with K/V AllGather.
Core c handles batch b = c // 4 and sequence chunk j = c % 4 (512 rows,
used both as its query block and as its K/V contribution).  Each core
projects K/V/Q only for its own 512 rows (3x less projection work than
computing K/V redundantly); K and V chunks are AllGather'd through HBM
bounce buffers within the 4-core batch group (replica groups [[0..3],
[4..7]], ~27 us per 4MB gather, overlapped with the remaining
projections).  Attention runs per head in scores.T [s, q] orientation
(exp on ACT with no max subtraction -- scores are ~N(0, 0.41)); softmax
denominators come free from a ones-column appended to V.  PV runs
transposed (es stationary, V moving, cost 65 rows instead of 512 per
s-tile) yielding A[q, d] plus the denominator column in PSUM; the
reciprocal denominator multiplies during eviction, and a PE transpose
returns A to [d, q] layout for the output projection.  Engine split: ACT
does only the 128 exp ops (the ~127 us floor), DVE does reciprocal/
normalize/evictions, Pool only triggers collectives (GPSIMD cannot touch
PSUM), PE does all matmuls/transposes.

Older zero-collective variants (build_mha/build_mha_v2) are kept for
reference and A/B timing.  Host-side prep passes operands pre-transposed
([in_dim, out_dim], contraction on partitions) and pre-rounded to bf16;
on-device accumulation is fp32 PSUM.

Collectives cannot live inside a For_i hardware loop (mesh desync), so
the reps>1 timing variant of v4 unrolls the body instead.
"""

import sys

for _p in ("/opt/trn_rl_repo",):
    if _p not in sys.path:
        sys.path.insert(0, _p)

import numpy as np
import ml_dtypes

import bass_rust
import concourse.bass as bass
import concourse.mybir as mybir
import concourse.tile as tile
from concourse.vector_clock import ScopedClock, VectorClock

F32 = mybir.dt.float32
F32R = mybir.dt.float32r
BF16 = mybir.dt.bfloat16
AF = mybir.ActivationFunctionType

D = 1024
S = 2048
SQ = 512
H = 16
DK = 64
NT_D = D // 128
NT_S = S // 128
NT_Q = SQ // 128
N_CORES = 8

# ---------------------------------------------------------------------------
# Workarounds for this walrus build, which accepts at most ONE semaphore wait
# per instruction ('Too many sync wait commands' in setupSyncWait).  Tile
# attaches multiple waits freely; split them across same-engine nops, and
# emit the kernel-tail drain one waited-semaphore at a time.
# ---------------------------------------------------------------------------

_WAITS_PER_INST = 1


def _split_drain_and_barrier(self, tick_clock, wait_clock):
    gc = tick_clock.global_clock
    n = len(gc)
    procs = [i for i in range(n) if gc[i] > 0]
    for i in range(0, len(procs), _WAITS_PER_INST):
        group = procs[i : i + _WAITS_PER_INST]
        vec = [0] * n
        for p in group:
            vec[p] = gc[p]
        drain_inst = self.nc.sync.drain()
        wait_clock.add_sem_waits(drain_inst.ins, ScopedClock({None: VectorClock(vec)}))

    self.nc.all_engine_barrier()
    assert self.sems is not None
    popped = self.nc._tile_sem_poison_stack.pop()
    assert popped is self._sem_poison
    self.nc.clear_and_free_semaphores(list(self.sems.allocated().values()))
    self.nc.all_engine_barrier()


tile.TileContext._drain_and_barrier = _split_drain_and_barrier


def _split_sync_waits(nc, limit=_WAITS_PER_INST):
    for f in nc.m.functions:
        for bb in f.blocks:
            insts = list(bb.instructions)
            if not any(
                inst.sync_info and len(inst.sync_info.on_wait or []) > limit
                for inst in insts
            ):
                continue
            new_list = []
            for inst in insts:
                si = inst.sync_info
                waits = list(si.on_wait) if si and si.on_wait else []
                if len(waits) > limit:
                    extra, keep = waits[:-limit], waits[-limit:]
                    for j in range(0, len(extra), limit):
                        chunk = extra[j : j + limit]
                        nop = nc.engines[inst.engine].nop(nofuse=True).ins
                        cur = nc.cur_bb.bb
                        assert cur.instructions[-1].name == nop.name
                        cur.instructions.pop()
                        nop.sync_info = bass_rust.SyncInfo(on_wait=chunk, on_update=[])
                        new_list.append(nop)
                    si.on_wait = keep
                new_list.append(inst)
            bb.instructions[:] = new_list


# ---------------------------------------------------------------------------
# v4: all-gather K/V builder
# ---------------------------------------------------------------------------
#
# Core c = (b, j): batch b = c//4, chunk j = c%4 owns sequence rows
# [512j, 512j+512) both as queries and as K/V rows.  Each core projects
# K/V/Q only for its own 512 rows; K and V chunks are AllGather'd (HBM
# bounce) within the 4-core batch group, giving full-S K/V with zero
# redundant projection work.  Attention runs per head with scores.T [s, q]
# (exp on ACT, no max subtraction), PV in transposed form (es stationary,
# V moving) yielding A[q, d] + denominator column; normalization multiplies
# by the reciprocal denominator during PSUM eviction, then a PE transpose
# puts A back in [d, q] layout for the output projection.
#
# Engine budget per core (rows = moving-row cycles at 128x128):
#   PE : 3x32768 proj + 131072 QK + 66560 PV + 8192 transp + 32768 Oproj
#   ACT: 128 exp ops of [128, 1024]  (the ~127 us floor; nothing else)
#   Pool: evictions w/ bias, normalize; DVE: recips, AT/y evictions.

GROUPS_V4 = [[0, 1, 2, 3], [4, 5, 6, 7]]


def build_mha_v4(reps=1, debug=False):
    nc = bass.Bass(num_devices=8)
    xTq_d = nc.declare_dram_parameter("xTq", [D, SQ], BF16, isOutput=False)
    WqT_d = nc.declare_dram_parameter("WqT", [D, D], BF16, isOutput=False)
    WkT_d = nc.declare_dram_parameter("WkT", [D, D], BF16, isOutput=False)
    WvT_d = nc.declare_dram_parameter("WvT", [D, D], BF16, isOutput=False)
    WoT_d = nc.declare_dram_parameter("WoT", [D, D], BF16, isOutput=False)
    bqt_d = nc.declare_dram_parameter("bqt", [128, NT_D], F32, isOutput=False)
    bkt_d = nc.declare_dram_parameter("bkt", [128, NT_D], F32, isOutput=False)
    bvr_d = nc.declare_dram_parameter("bvr", [1, D], BF16, isOutput=False)
    bor_d = nc.declare_dram_parameter("bor", [1, D], BF16, isOutput=False)
    ident_d = nc.declare_dram_parameter("ident", [128, 128], F32, isOutput=False)
    Y_d = nc.declare_dram_parameter("y", [SQ, D], F32, isOutput=True)
    if debug:
        KT_dbg = nc.declare_dram_parameter("kt_dbg", [128, NT_D, S], BF16, isOutput=True)
        VS_dbg = nc.declare_dram_parameter(
            "vs_dbg", [128, NT_S, H, DK + 1], BF16, isOutput=True
        )
        QT_dbg = nc.declare_dram_parameter("qt_dbg", [128, NT_D, SQ], BF16, isOutput=True)
        AT_dbg = nc.declare_dram_parameter("at_dbg", [128, NT_D, SQ], BF16, isOutput=True)

    wkT_r = WkT_d.rearrange("(c p) o -> p c o", p=128)
    wqT_r = WqT_d.rearrange("(c p) o -> p c o", p=128)
    NCH = 4  # chunks per batch group

    with tile.TileContext(nc) as tc:
        with (
            tc.tile_pool(name="persist", bufs=1) as persist,
            tc.tile_pool(name="stage", bufs=1) as stage,
            tc.tile_pool(name="wk", bufs=2) as wkp,
            tc.tile_pool(name="wq", bufs=2) as wqp,
            tc.tile_pool(name="es", bufs=3) as esp,
            tc.tile_pool(name="au", bufs=2) as aup,
            tc.tile_pool(name="small", bufs=2) as smallp,
            tc.tile_pool(name="ys", bufs=2) as ysp,
            tc.tile_pool(name="dram", bufs=1, space="DRAM") as dram,
            tc.tile_pool(name="ps_a", bufs=2, space="PSUM") as ps_a,
            tc.tile_pool(name="ps_qk", bufs=2, space="PSUM") as ps_qk,
            tc.tile_pool(name="ps_pv", bufs=2, space="PSUM") as ps_pv,
        ):
            KT = persist.tile([128, NT_D, S], BF16, name="KT")
            VS = persist.tile([128, NT_S, H, DK + 1], BF16, name="VS")
            QT = persist.tile([128, NT_D, SQ], BF16, name="QT")
            AT = persist.tile([128, NT_D, SQ], BF16, name="AT")
            bqt = persist.tile([128, NT_D], F32, name="bqt")
            bkt = persist.tile([128, NT_D], F32, name="bkt")
            bvb = persist.tile([128, D], BF16, name="bvb")
            bob = persist.tile([128, D], BF16, name="bob")
            ident = persist.tile([128, 128], F32, name="ident")

            xTq = stage.tile([128, NT_D, SQ], BF16, name="xTq")
            WvT = stage.tile([128, NT_D, D], BF16, name="WvT")
            WoT = stage.tile([128, NT_D, D], BF16, name="WoT")
            ko = stage.tile([128, NT_D, SQ], BF16, name="ko")
            vo = stage.tile([128, NT_S // NCH, H, DK + 1], BF16, name="vo")

            K_PIECES = [(0, 2), (2, 2), (4, 4)]  # (start_ot, n_ot)
            kk_ins = [
                dram.tile([128, n, SQ], BF16, name=f"kk_in_{i}")
                for i, (_, n) in enumerate(K_PIECES)
            ]
            kk_outs = [
                dram.tile([NCH, 128, n, SQ], BF16, name=f"kk_out_{i}")
                for i, (_, n) in enumerate(K_PIECES)
            ]
            vv_ins = [
                dram.tile([128, NT_S // NCH, H // 2, DK + 1], BF16, name=f"vv_in_{i}")
                for i in range(2)
            ]
            vv_outs = [
                dram.tile(
                    [NCH, 128, NT_S // NCH, H // 2, DK + 1], BF16, name=f"vv_out_{i}"
                )
                for i in range(2)
            ]

            def body():
                nc.sync.dma_start(out=xTq[:], in_=xTq_d.rearrange("(c p) s -> p c s", p=128))
                nc.sync.dma_start(out=bqt[:], in_=bqt_d[:])
                nc.sync.dma_start(out=bkt[:], in_=bkt_d[:])
                nc.sync.dma_start(out=bvb[:], in_=bvr_d.broadcast_to([128, D]))
                nc.sync.dma_start(out=bob[:], in_=bor_d.broadcast_to([128, D]))
                nc.sync.dma_start(out=ident[:], in_=ident_d[:])
                for dt in range(NT_D):
                    nc.sync.dma_start(
                        out=WvT[:, dt, :],
                        in_=WvT_d.rearrange("(c p) o -> p c o", p=128)[:, dt, :],
                    )

                # Phase A order: K(ot0-3) -> CC-K1 -> V -> CC-V -> Q(ot0-3)
                # -> K(ot4-7) -> CC-K2 -> Q(ot4-7).  Heads 0-7 only need the
                # first K half + first Q half, so attention (and the ACT exp
                # stream, the kernel's critical path) starts as soon as the
                # 2MB half-gather lands instead of waiting for all of K.  All
                # phase-A evictions run on DVE so ACT stays Exp-only (no
                # activation-table reloads mid-stream).

                def k_proj(ot):
                    wk = wkp.tile([128, NT_D, 128], BF16, name="wk")
                    nc.sync.dma_start(out=wk[:], in_=wkT_r[:, :, ot * 128 : (ot + 1) * 128])
                    pk = ps_a.tile([128, 512], F32, name="pa")
                    for dt in range(NT_D):
                        nc.tensor.matmul(
                            pk[:],
                            wk[:, dt, :],
                            xTq[:, dt, :],
                            start=(dt == 0),
                            stop=(dt == NT_D - 1),
                        )
                    nc.vector.tensor_scalar_add(ko[:, ot, :], pk[:], bkt[:, ot : ot + 1])
                    piece = 0 if ot < 2 else (1 if ot < 4 else 2)
                    s0 = K_PIECES[piece][0]
                    nc.sync.dma_start(out=kk_ins[piece][:, ot - s0, :], in_=ko[:, ot, :])

                def q_proj(ot):
                    wq = wqp.tile([128, NT_D, 128], BF16, name="wq")
                    nc.sync.dma_start(out=wq[:], in_=wqT_r[:, :, ot * 128 : (ot + 1) * 128])
                    pq = ps_a.tile([128, 512], F32, name="pa")
                    for dt in range(NT_D):
                        nc.tensor.matmul(
                            pq[:],
                            wq[:, dt, :],
                            xTq[:, dt, :],
                            start=(dt == 0),
                            stop=(dt == NT_D - 1),
                        )
                    nc.vector.tensor_scalar_add(QT[:, ot, :], pq[:], bqt[:, ot : ot + 1])

                def cc_k(piece):
                    s0, n = K_PIECES[piece]
                    nc.gpsimd.collective_compute(
                        "AllGather",
                        mybir.AluOpType.bypass,
                        replica_groups=GROUPS_V4,
                        ins=[kk_ins[piece].opt()],
                        outs=[kk_outs[piece].opt()],
                    )
                    for g in range(NCH):
                        nc.gpsimd.dma_start(
                            out=KT[:, s0 : s0 + n, g * SQ : (g + 1) * SQ],
                            in_=kk_outs[piece][g],
                        )

                def cc_v(oc):
                    nc.gpsimd.collective_compute(
                        "AllGather",
                        mybir.AluOpType.bypass,
                        replica_groups=GROUPS_V4,
                        ins=[vv_ins[oc].opt()],
                        outs=[vv_outs[oc].opt()],
                    )
                    for g in range(NCH):
                        nc.gpsimd.dma_start(
                            out=VS[:, g * 4 : (g + 1) * 4, oc * 8 : (oc + 1) * 8, :],
                            in_=vv_outs[oc][g],
                        )

                def v_proj(oc):
                    for st in range(NT_S // NCH):
                        pv = ps_a.tile([128, 512], F32, name="pa")
                        for dt in range(NT_D):
                            nc.tensor.matmul(
                                pv[:],
                                xTq[:, dt, st * 128 : (st + 1) * 128],
                                WvT[:, dt, oc * 512 : (oc + 1) * 512],
                                start=(dt == 0),
                                stop=(dt == NT_D - 1),
                            )
                        nc.vector.tensor_tensor(
                            out=vo[:, st, oc * 8 : (oc + 1) * 8, 0:DK],
                            in0=pv[:].rearrange("p (h d) -> p h d", d=DK),
                            in1=bvb[:, oc * 512 : (oc + 1) * 512].rearrange(
                                "p (h d) -> p h d", d=DK
                            ),
                            op=mybir.AluOpType.add,
                        )
                        nc.sync.dma_start(
                            out=vv_ins[oc][:, st],
                            in_=vo[:, st, oc * 8 : (oc + 1) * 8, :],
                        )

                nc.vector.memset(vo[:, :, :, DK : DK + 1], 1.0)
                for ot in range(2):
                    k_proj(ot)
                cc_k(0)
                for ot in range(2):
                    q_proj(ot)
                for ot in range(2, 4):
                    k_proj(ot)
                cc_k(1)
                v_proj(0)
                cc_v(0)
                v_proj(1)
                cc_v(1)
                for ot in range(2, 4):
                    q_proj(ot)
                for ot in range(4, NT_D):
                    k_proj(ot)
                cc_k(2)
                nc.sync.dma_start(out=WoT[:], in_=WoT_d.rearrange("(c p) o -> p c o", p=128))
                for ot in range(4, NT_D):
                    q_proj(ot)

                # ---- attention ----
                # Per head: 8 QK/exp units fill a whole-head es tile; PV then
                # runs one accumulation group per q-tile (PSUM allows a single
                # pending group per 2KB bank), interleaved into the NEXT
                # head's QK stream so ACT never starves.
                NSG = NT_S // 2  # 8 sg units per head
                es_tiles = {}
                au_tiles = {}

                def emit_qk(h, sg):
                    ot, po = h // 2, DK * (h % 2)
                    pqk = ps_qk.tile([128, 2, 512], F32, name="pqk")
                    for j in range(2):
                        st = 2 * sg + j
                        nc.tensor.matmul(
                            pqk[:, j, :],
                            KT[po : po + DK, ot, st * 128 : (st + 1) * 128],
                            QT[po : po + DK, ot, :],
                            start=True,
                            stop=True,
                        )
                    if sg == 0:
                        es_tiles[h] = esp.tile([128, NT_S, 512], BF16, name="es")
                    nc.scalar.activation(
                        es_tiles[h][:, 2 * sg : 2 * sg + 2, :].rearrange(
                            "p a b -> p (a b)"
                        ),
                        pqk[:].rearrange("p a b -> p (a b)"),
                        AF.Exp,
                        scale=0.125,
                    )

                def emit_pv(h, qt):
                    es = es_tiles[h]
                    ppv = ps_pv.tile([128, DK + 1], F32, name="ppv")
                    for st in range(NT_S):
                        nc.tensor.matmul(
                            ppv[:],
                            es[:, st, qt * 128 : (qt + 1) * 128],
                            VS[:, st, h, :],
                            start=(st == 0),
                            stop=(st == NT_S - 1),
                        )
                    if qt == NT_Q - 1:
                        es_tiles.pop(h)
                    r1 = smallp.tile([128, 1], F32, name="rQ")
                    nc.vector.reciprocal(out=r1[:], in_=ppv[:, DK : DK + 1])
                    if h % 2 == 0 and qt == 0:
                        au_tiles[h // 2] = aup.tile([128, NT_Q, 128], F32, name="au")
                    au = au_tiles[h // 2]
                    nc.vector.tensor_scalar(
                        out=au[:, qt, (h % 2) * DK : (h % 2) * DK + DK],
                        in0=ppv[:, 0:DK],
                        scalar1=r1[:],
                        scalar2=None,
                        op0=mybir.AluOpType.mult,
                    )

                def emit_transposes(ot):
                    au_done = au_tiles.pop(ot)
                    for qt in range(NT_Q):
                        pt = ps_a.tile([128, 128], F32, name="pa")
                        nc.tensor.transpose(pt[:], au_done[:, qt, :], ident[:])
                        nc.vector.tensor_copy(
                            out=AT[:, ot, qt * 128 : (qt + 1) * 128], in_=pt[:]
                        )

                # deferred-work queue: after qk unit k of head h, run pending
                # PV q-tile groups of head h-1 (slots after units 1..4) and
                # transposes of the completed pair (slot after unit 5).
                for h in range(H):
                    for sg in range(NSG):
                        emit_qk(h, sg)
                        if h >= 1 and 1 <= sg <= NT_Q:
                            emit_pv(h - 1, sg - 1)
                        if h >= 2 and h % 2 == 0 and sg == NT_Q + 1:
                            emit_transposes(h // 2 - 1)
                for qt in range(NT_Q):
                    emit_pv(H - 1, qt)
                emit_transposes(H // 2 - 1)

                # ---- output projection ----
                for qt in range(NT_Q):
                    for oc in range(2):
                        py = ps_pv.tile([128, 512], F32, name="ppv")
                        for ct in range(NT_D):
                            nc.tensor.matmul(
                                py[:],
                                AT[:, ct, qt * 128 : (qt + 1) * 128],
                                WoT[:, ct, oc * 512 : (oc + 1) * 512],
                                start=(ct == 0),
                                stop=(ct == NT_D - 1),
                            )
                        ys = ysp.tile([128, 512], F32, name="ys")
                        nc.vector.tensor_tensor(
                            out=ys[:],
                            in0=py[:],
                            in1=bob[:, oc * 512 : (oc + 1) * 512],
                            op=mybir.AluOpType.add,
                        )
                        nc.sync.dma_start(
                            out=Y_d[qt * 128 : (qt + 1) * 128, oc * 512 : (oc + 1) * 512],
                            in_=ys[:],
                        )
                if debug:
                    nc.sync.dma_start(out=KT_dbg[:], in_=KT[:])
                    nc.sync.dma_start(out=VS_dbg[:], in_=VS[:])
                    nc.sync.dma_start(out=QT_dbg[:], in_=QT[:])
                    nc.sync.dma_start(out=AT_dbg[:], in_=AT[:])

            # collectives are not usable inside For_i (mesh desync), so the
            # timing variant unrolls the body instead
            for _ in range(reps):
                body()

    _split_sync_waits(nc)
    return nc


def prep_core_inputs_v4(x, Wq, bq, Wk, bk, Wv, bv, Wo, bo):
    bf = ml_dtypes.bfloat16
    x = np.asarray(x, dtype=np.float32)
    WqT = np.ascontiguousarray(np.asarray(Wq).T).astype(bf)
    WkT = np.ascontiguousarray(np.asarray(Wk).T).astype(bf)
    WvT = np.ascontiguousarray(np.asarray(Wv).T).astype(bf)
    WoT = np.ascontiguousarray(np.asarray(Wo).T).astype(bf)
    bqt = np.ascontiguousarray(np.asarray(bq, dtype=np.float32).reshape(NT_D, 128).T)
    bkt = np.ascontiguousarray(np.asarray(bk, dtype=np.float32).reshape(NT_D, 128).T)
    bvr = np.asarray(bv).reshape(1, D).astype(bf)
    bor = np.asarray(bo).reshape(1, D).astype(bf)
    ident = np.eye(128, dtype=np.float32)
    in_maps = []
    for c in range(N_CORES):
        b, qc = c // 4, c % 4
        xTq = np.ascontiguousarray(x[b, qc * SQ : (qc + 1) * SQ].T).astype(bf)
        in_maps.append(
            dict(
                xTq=xTq, WqT=WqT, WkT=WkT, WvT=WvT, WoT=WoT,
                bqt=bqt, bkt=bkt, bvr=bvr, bor=bor, ident=ident,
            )
        )
    return in_maps


# ---------------------------------------------------------------------------
# Kernel builder
# ---------------------------------------------------------------------------


def build_mha_v2(reps=1, salt=0):
    """Interleaved variant: V projection first, then per-o-tile K/Q projection
    immediately followed by the two heads that consume that o-tile, so
    projection and attention overlap on all engines.  All pools co-resident
    (23 MB SBUF, 8 PSUM banks exactly); Wk/Wq streamed per o-tile.  V and y
    biases are added on DVE during PSUM eviction against DMA-broadcast bias
    tiles instead of K=1 matmuls.

    reps > 1 wraps the body in an on-device For_i loop (timing variant).
    """
    nc = bass.Bass()
    xT_d = nc.declare_dram_parameter("xT", [D, S], BF16, isOutput=False)
    xTq_d = nc.declare_dram_parameter("xTq", [D, SQ], BF16, isOutput=False)
    WqT_d = nc.declare_dram_parameter("WqT", [D, D], BF16, isOutput=False)
    WkT_d = nc.declare_dram_parameter("WkT", [D, D], BF16, isOutput=False)
    WvT_d = nc.declare_dram_parameter("WvT", [D, D], BF16, isOutput=False)
    WoT_d = nc.declare_dram_parameter("WoT", [D, D], BF16, isOutput=False)
    bqt_d = nc.declare_dram_parameter("bqt", [128, NT_D], F32, isOutput=False)
    bkt_d = nc.declare_dram_parameter("bkt", [128, NT_D], F32, isOutput=False)
    bvr_d = nc.declare_dram_parameter("bvr", [1, D], BF16, isOutput=False)
    bor_d = nc.declare_dram_parameter("bor", [1, D], BF16, isOutput=False)
    Y_d = nc.declare_dram_parameter("y", [SQ, D], F32, isOutput=True)

    wkT_r = WkT_d.rearrange("(c p) o -> p c o", p=128)
    wqT_r = WqT_d.rearrange("(c p) o -> p c o", p=128)

    with tile.TileContext(nc) as tc:
        with (
            tc.tile_pool(name="persist", bufs=1) as persist,
            tc.tile_pool(name="ph1", bufs=1) as ph1,
            tc.tile_pool(name="wk", bufs=2) as wkp,
            tc.tile_pool(name="wq", bufs=2) as wqp,
            tc.tile_pool(name="es", bufs=3) as esp,
            tc.tile_pool(name="small", bufs=2) as small,
            tc.tile_pool(name="ph3", bufs=1) as ph3,
            tc.tile_pool(name="ps_big", bufs=2, space="PSUM") as ps_big,
            tc.tile_pool(name="ps_mm", bufs=2, space="PSUM") as ps_mm,
            tc.tile_pool(name="ps_pv", bufs=2, space="PSUM") as ps_pv,
        ):
            KT = persist.tile([128, NT_D, S], BF16, name="KT")
            QT = persist.tile([128, NT_D, SQ], BF16, name="QT")
            VS = persist.tile([128, NT_S, H, DK + 1], BF16, name="VS")
            AT = persist.tile([128, NT_D, SQ], BF16, name="AT")
            bqt = persist.tile([128, NT_D], F32, name="bqt")
            bkt = persist.tile([128, NT_D], F32, name="bkt")
            bvb = persist.tile([128, D], BF16, name="bvb")
            bob = persist.tile([128, D], BF16, name="bob")
            ones64r = persist.tile([1, DK], F32R, name="ones64r")
            ones64f = persist.tile([1, DK], F32, name="ones64f")

            xT = ph1.tile([128, NT_D, S], BF16, name="xT")
            xTq = ph1.tile([128, NT_D, SQ], BF16, name="xTq")
            WvT = ph1.tile([128, NT_D, D], BF16, name="WvT")
            WoT = ph3.tile([128, NT_D, D], BF16, name="WoT")

            def body():
                # V operands first -- V production is the kernel's opening act
                for dt in range(NT_D):
                    nc.sync.dma_start(
                        out=WvT[:, dt, :],
                        in_=WvT_d.rearrange("(c p) o -> p c o", p=128)[:, dt, :],
                    )
                for sc in range(4):
                    nc.sync.dma_start(
                        out=xT[:, :, sc * 512 : (sc + 1) * 512],
                        in_=xT_d.rearrange("(c p) s -> p c s", p=128)[
                            :, :, sc * 512 : (sc + 1) * 512
                        ],
                    )
                nc.sync.dma_start(out=xTq[:], in_=xTq_d.rearrange("(c p) s -> p c s", p=128))
                nc.sync.dma_start(out=bqt[:], in_=bqt_d[:])
                nc.sync.dma_start(out=bkt[:], in_=bkt_d[:])
                # partition-broadcast bias rows (DRAM source, partition step 0)
                nc.sync.dma_start(out=bvb[:], in_=bvr_d.broadcast_to([128, D]))
                nc.sync.dma_start(out=bob[:], in_=bor_d.broadcast_to([128, D]))
                nc.vector.memset(ones64f[:], 1.0)
                nc.vector.tensor_copy(out=ones64r[:], in_=ones64f[:])
                nc.vector.memset(VS[:, :, :, DK : DK + 1], 1.0)
                nc.sync.dma_start(out=WoT[:], in_=WoT_d.rearrange("(c p) o -> p c o", p=128))

                # ---- V production ----
                for st in range(NT_S):
                    for oc in range(2):
                        pv = ps_mm.tile([128, 512], F32, name="mm")
                        for dt in range(NT_D):
                            nc.tensor.matmul(
                                pv[:],
                                xT[:, dt, st * 128 : (st + 1) * 128],
                                WvT[:, dt, oc * 512 : (oc + 1) * 512],
                                start=(dt == 0),
                                stop=(dt == NT_D - 1),
                            )
                        nc.vector.tensor_tensor(
                            out=VS[:, st, oc * 8 : (oc + 1) * 8, 0:DK],
                            in0=pv[:].rearrange("p (h d) -> p h d", d=DK),
                            in1=bvb[:, oc * 512 : (oc + 1) * 512].rearrange(
                                "p (h d) -> p h d", d=DK
                            ),
                            op=mybir.AluOpType.add,
                        )

                # ---- per-o-tile projection + the two heads that consume it ----
                for ot in range(NT_D):
                    wk = wkp.tile([128, NT_D, 128], BF16, name="wk")
                    nc.sync.dma_start(out=wk[:], in_=wkT_r[:, :, ot * 128 : (ot + 1) * 128])
                    for half in range(2):
                        pk2 = ps_big.tile([128, 2, 512], F32, name="big2")
                        for sc in range(2):
                            for dt in range(NT_D):
                                nc.tensor.matmul(
                                    pk2[:, sc, :],
                                    wk[:, dt, :],
                                    xT[:, dt, (2 * half + sc) * 512 : (2 * half + sc + 1) * 512],
                                    start=(dt == 0),
                                    stop=(dt == NT_D - 1),
                                )
                        nc.scalar.activation(
                            KT[:, ot, half * 1024 : (half + 1) * 1024],
                            pk2[:, :, :].rearrange("p a b -> p (a b)"),
                            AF.Identity,
                            bias=bkt[:, ot : ot + 1],
                        )
                    wq = wqp.tile([128, NT_D, 128], BF16, name="wq")
                    nc.sync.dma_start(out=wq[:], in_=wqT_r[:, :, ot * 128 : (ot + 1) * 128])
                    pq = ps_mm.tile([128, 512], F32, name="mm")
                    for dt in range(NT_D):
                        nc.tensor.matmul(
                            pq[:],
                            wq[:, dt, :],
                            xTq[:, dt, :],
                            start=(dt == 0),
                            stop=(dt == NT_D - 1),
                        )
                    nc.scalar.activation(
                        QT[:, ot, :], pq[:], AF.Identity, bias=bqt[:, ot : ot + 1]
                    )

                    for h in (2 * ot, 2 * ot + 1):
                        po = DK * (h % 2)
                        ppv = ps_pv.tile([DK + 1, 512], F32, name="ppv")
                        for half in range(2):
                            es = esp.tile([128, NT_S // 2, 512], BF16, name="es")
                            for sg in range(4):
                                pqk = ps_big.tile([128, 2, 512], F32, name="big2")
                                for j in range(2):
                                    st = half * 8 + 2 * sg + j
                                    nc.tensor.matmul(
                                        pqk[:, j, :],
                                        KT[po : po + DK, ot, st * 128 : (st + 1) * 128],
                                        QT[po : po + DK, ot, :],
                                        start=True,
                                        stop=True,
                                    )
                                nc.scalar.activation(
                                    es[:, 2 * sg : 2 * sg + 2, :].rearrange(
                                        "p a b -> p (a b)"
                                    ),
                                    pqk[:].rearrange("p a b -> p (a b)"),
                                    AF.Exp,
                                    scale=0.125,
                                )
                            for j in range(NT_S // 2):
                                st = half * 8 + j
                                nc.tensor.matmul(
                                    ppv[:],
                                    VS[:, st, h, :],
                                    es[:, j, :],
                                    start=(st == 0),
                                    stop=(st == NT_S - 1),
                                )
                        r = small.tile([1, 512], F32R, name="r")
                        with nc.allow_low_precision(reason="f32r recip, 1.2e-4 rel"):
                            nc.vector.reciprocal(out=r[:], in_=ppv[DK : DK + 1, :])
                        prb = ps_mm.tile([DK, 512], F32, name="mm")
                        nc.tensor.matmul(prb[:], ones64r[:], r[:], start=True, stop=True)
                        au = small.tile([DK, 512], F32, name="au")
                        nc.vector.tensor_copy(out=au[:], in_=ppv[0:DK, :])
                        nc.vector.tensor_mul(
                            out=AT[po : po + DK, ot, :], in0=au[:], in1=prb[:]
                        )

                # ---- output projection ----
                for qt in range(NT_Q):
                    for oc in range(2):
                        py = ps_big.tile([128, 512], F32, name="big2")
                        for ct in range(NT_D):
                            nc.tensor.matmul(
                                py[:],
                                AT[:, ct, qt * 128 : (qt + 1) * 128],
                                WoT[:, ct, oc * 512 : (oc + 1) * 512],
                                start=(ct == 0),
                                stop=(ct == NT_D - 1),
                            )
                        ys = small.tile([128, 512], F32, name="au")
                        nc.vector.tensor_tensor(
                            out=ys[:],
                            in0=py[:],
                            in1=bob[:, oc * 512 : (oc + 1) * 512],
                            op=mybir.AluOpType.add,
                        )
                        nc.sync.dma_start(
                            out=Y_d[qt * 128 : (qt + 1) * 128, oc * 512 : (oc + 1) * 512],
                            in_=ys[:],
                        )


            if reps > 1:
                with tc.For_i(0, reps, 1):
                    body()
            else:
                body()

    _split_sync_waits(nc)
    return nc


def build_mha(reps=1):
    """reps > 1 wraps the body in an on-device For_i loop (timing variant)."""
    nc = bass.Bass()
    xT_d = nc.declare_dram_parameter("xT", [D, S], BF16, isOutput=False)
    xTq_d = nc.declare_dram_parameter("xTq", [D, SQ], BF16, isOutput=False)
    WqT_d = nc.declare_dram_parameter("WqT", [D, D], BF16, isOutput=False)
    WkT_d = nc.declare_dram_parameter("WkT", [D, D], BF16, isOutput=False)
    WvT_d = nc.declare_dram_parameter("WvT", [D, D], BF16, isOutput=False)
    WoT_d = nc.declare_dram_parameter("WoT", [D, D], BF16, isOutput=False)
    bqt_d = nc.declare_dram_parameter("bqt", [128, NT_D], F32, isOutput=False)
    bkt_d = nc.declare_dram_parameter("bkt", [128, NT_D], F32, isOutput=False)
    bvr_d = nc.declare_dram_parameter("bvr", [1, D], BF16, isOutput=False)
    bor_d = nc.declare_dram_parameter("bor", [1, D], BF16, isOutput=False)
    Y_d = nc.declare_dram_parameter("y", [SQ, D], F32, isOutput=True)

    with tile.TileContext(nc) as tc:
        with tc.tile_pool(name="persist", bufs=1) as persist:
            KT = persist.tile([128, NT_D, S], BF16, name="KT")
            QT = persist.tile([128, NT_D, SQ], BF16, name="QT")
            VS = persist.tile([128, NT_S, H, DK + 1], BF16, name="VS")
            AT = persist.tile([128, NT_D, SQ], BF16, name="AT")
            bqt = persist.tile([128, NT_D], F32, name="bqt")
            bkt = persist.tile([128, NT_D], F32, name="bkt")
            bvr = persist.tile([1, D], BF16, name="bvr")
            bor = persist.tile([1, D], BF16, name="bor")
            ones1 = persist.tile([1, 128], BF16, name="ones1")
            ones64r = persist.tile([1, DK], F32R, name="ones64r")
            ones64f = persist.tile([1, DK], F32, name="ones64f")

            nc.sync.dma_start(out=bqt[:], in_=bqt_d[:])
            nc.sync.dma_start(out=bkt[:], in_=bkt_d[:])
            nc.sync.dma_start(out=bvr[:], in_=bvr_d[:])
            nc.sync.dma_start(out=bor[:], in_=bor_d[:])
            nc.vector.memset(ones1[:], 1.0)
            nc.vector.memset(ones64f[:], 1.0)
            nc.vector.tensor_copy(out=ones64r[:], in_=ones64f[:])
            nc.vector.memset(VS[:, :, :, DK : DK + 1], 1.0)

            def phase1():
                with (
                    tc.tile_pool(name="ph1", bufs=1) as ph1,
                    tc.tile_pool(name="ps_kt", bufs=1, space="PSUM") as ps_kt,
                    tc.tile_pool(name="ps_mm", bufs=2, space="PSUM") as ps_mm,
                ):
                    xT = ph1.tile([128, NT_D, S], BF16, name="xT")
                    xTq = ph1.tile([128, NT_D, SQ], BF16, name="xTq")
                    WkT = ph1.tile([128, NT_D, D], BF16, name="WkT")
                    WqT = ph1.tile([128, NT_D, D], BF16, name="WqT")
                    WvT = ph1.tile([128, NT_D, D], BF16, name="WvT")
                    nc.sync.dma_start(
                        out=xT[:], in_=xT_d.rearrange("(c p) s -> p c s", p=128)
                    )
                    nc.sync.dma_start(
                        out=WkT[:], in_=WkT_d.rearrange("(c p) o -> p c o", p=128)
                    )
                    nc.sync.dma_start(
                        out=xTq[:], in_=xTq_d.rearrange("(c p) s -> p c s", p=128)
                    )
                    nc.sync.dma_start(
                        out=WqT[:], in_=WqT_d.rearrange("(c p) o -> p c o", p=128)
                    )
                    nc.sync.dma_start(
                        out=WvT[:], in_=WvT_d.rearrange("(c p) o -> p c o", p=128)
                    )

                    for ot in range(NT_D):
                        # K.T block [o-tile, all s] in a 4-bank psum tile
                        pk = ps_kt.tile([128, 4, 512], F32, name="pk")
                        for sc in range(4):
                            for dt in range(NT_D):
                                nc.tensor.matmul(
                                    pk[:, sc, :],
                                    WkT[:, dt, ot * 128 : (ot + 1) * 128],
                                    xT[:, dt, sc * 512 : (sc + 1) * 512],
                                    start=(dt == 0),
                                    stop=(dt == NT_D - 1),
                                )
                        nc.scalar.activation(
                            KT[:, ot, :],
                            pk[:, :, :].rearrange("p a b -> p (a b)"),
                            AF.Identity,
                            bias=bkt[:, ot : ot + 1],
                        )

                        # V for two s-tiles (fills PE while ACT evicts K.T)
                        for st in (2 * ot, 2 * ot + 1):
                            for oc in range(2):
                                pv = ps_mm.tile([128, 512], F32, name="mm")
                                for dt in range(NT_D):
                                    nc.tensor.matmul(
                                        pv[:],
                                        xT[:, dt, st * 128 : (st + 1) * 128],
                                        WvT[:, dt, oc * 512 : (oc + 1) * 512],
                                        start=(dt == 0),
                                        stop=False,
                                    )
                                nc.tensor.matmul(
                                    pv[:],
                                    ones1[:],
                                    bvr[:, oc * 512 : (oc + 1) * 512],
                                    start=False,
                                    stop=True,
                                )
                                nc.vector.tensor_copy(
                                    out=VS[:, st, oc * 8 : (oc + 1) * 8, 0:DK],
                                    in_=pv[:].rearrange("p (h d) -> p h d", d=DK),
                                )

                        # Q.T block
                        pq = ps_mm.tile([128, 512], F32, name="mm")
                        for dt in range(NT_D):
                            nc.tensor.matmul(
                                pq[:],
                                WqT[:, dt, ot * 128 : (ot + 1) * 128],
                                xTq[:, dt, :],
                                start=(dt == 0),
                                stop=(dt == NT_D - 1),
                            )
                        nc.scalar.activation(
                            QT[:, ot, :], pq[:], AF.Identity, bias=bqt[:, ot : ot + 1]
                        )

            def phase23():
                with (
                    tc.tile_pool(name="ph2", bufs=1) as ph2,
                    tc.tile_pool(name="es", bufs=3) as esp,
                    tc.tile_pool(name="small", bufs=2) as small,
                    tc.tile_pool(name="ps_qk", bufs=2, space="PSUM") as ps_qk,
                    tc.tile_pool(name="ps_pv", bufs=2, space="PSUM") as ps_pv,
                    tc.tile_pool(name="ps_rb", bufs=1, space="PSUM") as ps_rb,
                ):
                    WoT = ph2.tile([128, NT_D, D], BF16, name="WoT")
                    nc.sync.dma_start(
                        out=WoT[:], in_=WoT_d.rearrange("(c p) o -> p c o", p=128)
                    )

                    for h in range(H):
                        ot, po = h // 2, DK * (h % 2)
                        es = esp.tile([128, NT_S, 512], BF16, name="es")
                        for sg in range(NT_S // 2):
                            pqk = ps_qk.tile([128, 2, 512], F32, name="pqk")
                            for j in range(2):
                                st = 2 * sg + j
                                nc.tensor.matmul(
                                    pqk[:, j, :],
                                    KT[po : po + DK, ot, st * 128 : (st + 1) * 128],
                                    QT[po : po + DK, ot, :],
                                    start=True,
                                    stop=True,
                                )
                            nc.scalar.activation(
                                es[:, 2 * sg : 2 * sg + 2, :].rearrange(
                                    "p a b -> p (a b)"
                                ),
                                pqk[:].rearrange("p a b -> p (a b)"),
                                AF.Exp,
                                scale=0.125,
                            )
                        ppv = ps_pv.tile([DK + 1, 512], F32, name="ppv")
                        for st in range(NT_S):
                            nc.tensor.matmul(
                                ppv[:],
                                VS[:, st, h, :],
                                es[:, st, :],
                                start=(st == 0),
                                stop=(st == NT_S - 1),
                            )
                        r = small.tile([1, 512], F32R, name="r")
                        with nc.allow_low_precision(reason="f32r recip, 1.2e-4 rel"):
                            nc.vector.reciprocal(out=r[:], in_=ppv[DK : DK + 1, :])
                        prb = ps_rb.tile([DK, 512], F32, name="prb")
                        nc.tensor.matmul(prb[:], ones64r[:], r[:], start=True, stop=True)
                        au = small.tile([DK, 512], F32, name="au")
                        nc.vector.tensor_copy(out=au[:], in_=ppv[0:DK, :])
                        nc.vector.tensor_mul(
                            out=AT[po : po + DK, ot, :], in0=au[:], in1=prb[:]
                        )

                    # ---- phase 3: output projection ----
                    for qt in range(NT_Q):
                        for oc in range(2):
                            py = ps_pv.tile([128, 512], F32, name="ppv")
                            for ct in range(NT_D):
                                nc.tensor.matmul(
                                    py[:],
                                    AT[:, ct, qt * 128 : (qt + 1) * 128],
                                    WoT[:, ct, oc * 512 : (oc + 1) * 512],
                                    start=(ct == 0),
                                    stop=False,
                                )
                            nc.tensor.matmul(
                                py[:],
                                ones1[:],
                                bor[:, oc * 512 : (oc + 1) * 512],
                                start=False,
                                stop=True,
                            )
                            ys = small.tile([128, 512], F32, name="ys")
                            nc.vector.tensor_copy(out=ys[:], in_=py[:])
                            nc.sync.dma_start(
                                out=Y_d[
                                    qt * 128 : (qt + 1) * 128,
                                    oc * 512 : (oc + 1) * 512,
                                ],
                                in_=ys[:],
                            )

            if reps > 1:
                with tc.For_i(0, reps, 1):
                    phase1()
                    phase23()
            else:
                phase1()
                phase23()

    _split_sync_waits(nc)
    return nc


# ---------------------------------------------------------------------------
# Host-side sharding / unsharding
# ---------------------------------------------------------------------------


def prep_core_inputs(x, Wq, bq, Wk, bk, Wv, bv, Wo, bo):
    bf = ml_dtypes.bfloat16
    x = np.asarray(x, dtype=np.float32)
    WqT = np.ascontiguousarray(np.asarray(Wq).T).astype(bf)
    WkT = np.ascontiguousarray(np.asarray(Wk).T).astype(bf)
    WvT = np.ascontiguousarray(np.asarray(Wv).T).astype(bf)
    WoT = np.ascontiguousarray(np.asarray(Wo).T).astype(bf)
    bqt = np.ascontiguousarray(np.asarray(bq, dtype=np.float32).reshape(NT_D, 128).T)
    bkt = np.ascontiguousarray(np.asarray(bk, dtype=np.float32).reshape(NT_D, 128).T)
    bvr = np.asarray(bv).reshape(1, D).astype(bf)
    bor = np.asarray(bo).reshape(1, D).astype(bf)
    in_maps = []
    for c in range(N_CORES):
        b, qc = c // 4, c % 4
        xb = x[b]
        xT = np.ascontiguousarray(xb.T).astype(bf)
        xTq = np.ascontiguousarray(xb[qc * SQ : (qc + 1) * SQ].T).astype(bf)
        in_maps.append(
            dict(
                xT=xT, xTq=xTq, WqT=WqT, WkT=WkT, WvT=WvT, WoT=WoT,
                bqt=bqt, bkt=bkt, bvr=bvr, bor=bor,
            )
        )
    return in_maps


def assemble_output(outs):
    y = np.empty((2, S, D), dtype=np.float32)
    for c in range(N_CORES):
        b, qc = c // 4, c % 4
        y[b, qc * SQ : (qc + 1) * SQ, :] = outs[c]["y"]
    return y


_NC_CACHE = {}


def kernel(**inputs) -> np.ndarray:
    import time

    from concourse.bass_utils import run_bass_kernel_spmd

    if "nc" not in _NC_CACHE:
        _NC_CACHE["nc"] = build_mha_v4()
    nc = _NC_CACHE["nc"]
    in_maps = prep_core_inputs_v4(**inputs)
    # The tunnel-attached device occasionally reports
    # NRT_EXEC_UNIT_UNRECOVERABLE right after a prior heavy run; it recovers
    # on its own within ~90 s.  Retry once before giving up.
    try:
        res = run_bass_kernel_spmd(nc, in_maps, core_ids=list(range(N_CORES)))
    except Exception:
        time.sleep(90)
        res = run_bass_kernel_spmd(nc, in_maps, core_ids=list(range(N_CORES)))
    return assemble_output(res.results)

